# revision 6
# baseline (speedup 1.0000x reference)
"""2-layer GAT (PyG GATConv semantics) on 8 Trainium2 NeuronCores.

Sharding: edges partitioned by destination node (12500 dst nodes per core).
Per layer, each core:
  - builds the full node table rows [h | as | ad | pad] (512B) in its own HBM
    via PE matmul from the replicated layer input (x^T resp. out1^T),
  - edge phase: dst nodes in fixed groups of 128 (partition-aligned); each
    group's incoming edges laid out round-major per src-window (4 windows of
    25088 table rows so dma_gather's int16 indices suffice); 512B rows
    gathered per edge; softmax num/den accumulated with DVE mult+reduce.
    No max-subtraction is needed: e values are O(1) so exp() cannot overflow
    and softmax is shift-invariant. Dummy edge slots index a dummy row with
    a_src=-100 => exp contribution ~4e-8.
Two NEFF dispatches (one per GAT layer); host only reshards between them.
"""
import numpy as np

N = 100000
F_IN = 128
HID = 64
HEADS = 4
EPS = 1e-16
SLOPE = 0.2
NC = 8
PER = N // NC
NGRP = (PER + 127) // 128      # 98
NPAD = NGRP * 128              # 12544
NWIN = 4
WCAP = 25087
WROW = 25088
V = NWIN * WROW
D = 128                        # table row width in fp32 (512B)
AS_DUMMY = -100.0

_cache = {}


def _split_waits(nc):
    """This walrus encodes at most one sync-wait per instruction; move the
    excess onto InstNoOps just before the instruction (same engine order)."""
    import bass_rust
    import concourse.mybir as mybir
    n = 0
    for f in nc.m.functions:
        for bb in f.blocks:
            insts = bb.instructions
            out = []
            changed = False
            for ins in insts:
                si = ins.sync_info
                if si is not None and len(si.on_wait) > 1:
                    waits = list(si.on_wait)
                    for j, w in enumerate(waits[:-1]):
                        nop = mybir.InstNoOp(name=f"{ins.name}-ws{j}",
                                             engine=ins.engine, ins=[], outs=[])
                        nop.sync_info = bass_rust.SyncInfo(on_wait=[w],
                                                           on_update=[])
                        out.append(nop)
                    ins.sync_info = bass_rust.SyncInfo(
                        on_wait=waits[-1:], on_update=list(si.on_update))
                    changed = True
                    n += 1
                out.append(ins)
            if changed:
                bb.instructions = out
    return n


def _prep(edge_index):
    import hashlib
    key = hashlib.sha1(np.ascontiguousarray(edge_index)).hexdigest()
    if key in _cache:
        return _cache[key]
    src = np.asarray(edge_index[0], np.int64)
    dst = np.asarray(edge_index[1], np.int64)
    wsrc = np.minimum(src // WCAP, NWIN - 1)
    lsrc = src - wsrc * WCAP
    core = dst // PER
    grp = (dst % PER) // 128
    part = (dst % PER) % 128
    okey = ((core * NWIN + wsrc) * NGRP + grp) * 128 + part
    order = np.argsort(okey, kind="stable")
    ok_s = okey[order]
    ls_s = lsrc[order]
    first = np.r_[0, np.flatnonzero(np.diff(ok_s)) + 1]
    runlen = np.diff(np.r_[first, len(ok_s)])
    rank = np.arange(len(ok_s)) - np.repeat(first, runlen)
    cntd = np.zeros(NC * NWIN * NGRP * 128, np.int64)
    cntd[ok_s[first]] = runlen
    cntd = cntd.reshape(NC, NWIN, NGRP, 128)
    rounds = cntd.max(axis=3).max(axis=0).astype(np.int64)   # [NWIN, NGRP]
    tile_base = np.zeros((NWIN, NGRP), np.int64)
    for w in range(NWIN):
        tile_base[w, 1:] = np.cumsum(rounds[w])[:-1]
    c_s = ok_s // (NWIN * NGRP * 128)
    w_s = (ok_s // (NGRP * 128)) % NWIN
    g_s = (ok_s // 128) % NGRP
    p_s = ok_s % 128
    t_s = tile_base[w_s, g_s] + rank
    idx_arrays = []
    for c in range(NC):
        per_w = []
        for w in range(NWIN):
            ntile = int(rounds[w].sum())
            arr = np.full((ntile, 128), WCAP, np.int16)
            m = (c_s == c) & (w_s == w)
            arr[t_s[m], p_s[m]] = ls_s[m].astype(np.int16)
            wr = arr.reshape(ntile, 8, 16).transpose(2, 0, 1).reshape(16, ntile * 8)
            per_w.append(np.ascontiguousarray(np.tile(wr, (8, 1)), dtype=np.int16))
        idx_arrays.append(per_w)
    res = (rounds, idx_arrays)
    _cache[key] = res
    return res


def _build_layer(rounds, f_in, heads, last):
    import concourse.bacc as bacc
    import concourse.mybir as mybir
    from concourse.tile import TileContext

    H = heads
    CH = 64
    f32 = mybir.dt.float32
    AT = mybir.AluOpType
    nc = bacc.Bacc("TRN2")
    nfull = ((N + 127) // 128) * 128
    xT = nc.dram_tensor("xT", [f_in, nfull], f32, kind="ExternalInput")
    xsT = nc.dram_tensor("xsT", [f_in, NPAD], f32, kind="ExternalInput")
    Wcomb = nc.dram_tensor("Wcomb", [f_in, CH + 2 * H], f32, kind="ExternalInput")
    dumrow = nc.dram_tensor("dumrow", [NWIN, D], f32, kind="ExternalInput")
    bias = nc.dram_tensor("bias", [128, CH], f32, kind="ExternalInput")
    wcf = nc.dram_tensor("wcf", [128, CH], f32, kind="ExternalInput")
    idxs = [nc.dram_tensor(f"idx{w}", [128, int(rounds[w].sum()) * 8],
                           mybir.dt.int16, kind="ExternalInput")
            for w in range(NWIN)]
    table = nc.dram_tensor("table", [V, D], f32)
    out = nc.dram_tensor("out", [PER, 1 if last else CH], f32,
                         kind="ExternalOutput")

    with TileContext(nc) as tc:
        with tc.tile_pool(name="const", bufs=1) as cpool, \
             tc.tile_pool(name="work", bufs=3) as pool, \
             tc.tile_pool(name="gb", bufs=3) as gpool, \
             tc.tile_pool(name="acc", bufs=2) as apool, \
             tc.tile_pool(name="ps", bufs=4, space="PSUM") as ppool, \
             tc.tile_pool(name="adp", bufs=1) as adpool:
            wct = cpool.tile([f_in, CH + 2 * H], f32)
            nc.sync.dma_start(out=wct[:], in_=Wcomb[:, :])
            bt = cpool.tile([128, CH], f32)
            nc.sync.dma_start(out=bt[:], in_=bias[:, :])
            wcft = cpool.tile([128, CH], f32)
            nc.sync.dma_start(out=wcft[:], in_=wcf[:, :])
            ad_arr = adpool.tile([128, NGRP * H], f32)

            table_writes = []
            for w in range(NWIN):
                dt_ = pool.tile([1, D], f32, tag="dt")
                nc.sync.dma_start(out=dt_[:], in_=dumrow[w:w + 1, :])
                table_writes.append(
                    nc.sync.dma_start(out=table[(w + 1) * WROW - 1:(w + 1) * WROW, :],
                                      in_=dt_[:]))

            # full-table prologue
            for ci in range(nfull // 128):
                n0 = ci * 128
                lx = pool.tile([f_in, 128], f32, tag="lx")
                nc.sync.dma_start(out=lx[:], in_=xT[:, n0:n0 + 128])
                ps = ppool.tile([128, CH + 2 * H], f32)
                nc.tensor.matmul(ps[:], lhsT=lx[:], rhs=wct[:],
                                 start=True, stop=True)
                st = pool.tile([128, D], f32, tag="st")
                nc.scalar.copy(out=st[:, :CH + H], in_=ps[:, :CH + H])
                runs = []
                a = n0
                while a < n0 + 128:
                    w = min(a // WCAP, NWIN - 1)
                    b = min(n0 + 128, (w + 1) * WCAP) if w < NWIN - 1 else n0 + 128
                    runs.append((a, b, a + w))
                    a = b
                for (a, b, row) in runs:
                    table_writes.append(
                        nc.sync.dma_start(out=table[row:row + (b - a), :],
                                          in_=st[a - n0:b - n0, :]))

            # per-core ad (own dst slice)
            for g in range(NGRP):
                lxs = pool.tile([f_in, 128], f32, tag="lxs")
                nc.sync.dma_start(out=lxs[:], in_=xsT[:, g * 128:(g + 1) * 128])
                ps2 = ppool.tile([128, H], f32, tag="ps2")
                nc.tensor.matmul(ps2[:], lhsT=lxs[:], rhs=wct[:, CH + H:CH + 2 * H],
                                 start=True, stop=True)
                nc.vector.tensor_copy(out=ad_arr[:, g * H:(g + 1) * H], in_=ps2[:])

            # fence: Tile does not track DRAM RAW deps; gathers must wait
            # for all table writes.
            from concourse.tile_rust import add_dep_helper
            fence_t = pool.tile([1, 4], f32, tag="fence")
            fence = nc.vector.memset(fence_t[:], 0.0)
            for wd in table_writes:
                add_dep_helper(fence.ins, wd.ins, reason="table RAW fence")

            # edge phase: per group, per window band
            tile_base = np.zeros((NWIN, NGRP), np.int64)
            for w in range(NWIN):
                tile_base[w, 1:] = np.cumsum(rounds[w])[:-1]
            for g in range(NGRP):
                acc = apool.tile([128, CH + H], f32, tag="acc")
                nc.vector.memset(acc[:], 0.0)
                for w in range(NWIN):
                    r = int(rounds[w, g])
                    if r == 0:
                        continue
                    t0 = int(tile_base[w, g])
                    it = pool.tile([128, r * 8], mybir.dt.int16, tag="it")
                    nc.sync.dma_start(out=it[:], in_=idxs[w][:, t0 * 8:(t0 + r) * 8])
                    gb = gpool.tile([128, r, D], f32, tag="gb")
                    gth = nc.gpsimd.dma_gather(gb[:], table[w * WROW:(w + 1) * WROW, :],
                                               it[:], r * 128, r * 128, D,
                                               single_packet=False)
                    add_dep_helper(gth.ins, fence.ins, reason="table RAW fence")
                    ex = pool.tile([128, H, r], f32, tag="ex")
                    for h in range(H):
                        nc.vector.tensor_tensor(
                            out=ex[:, h:h + 1, :],
                            in0=gb[:, :, CH + h:CH + h + 1].rearrange("p j c -> p c j"),
                            in1=ad_arr[:, g * H + h:g * H + h + 1][:, :, None]
                                .to_broadcast([128, 1, r]),
                            op=AT.add)
                    exf = ex[:].rearrange("p h j -> p (h j)")
                    lr = pool.tile([128, H * r], f32, tag="lr")
                    nc.vector.tensor_scalar_mul(lr[:], exf, SLOPE)
                    nc.vector.tensor_tensor(out=lr[:], in0=lr[:], in1=exf, op=AT.max)
                    nc.scalar.activation(exf, lr[:], mybir.ActivationFunctionType.Exp)
                    m = pool.tile([128, CH, r], f32, tag="m")
                    chh = CH // H
                    for h in range(H):
                        nc.vector.tensor_tensor(
                            out=m[:, h * chh:(h + 1) * chh, :],
                            in0=gb[:, :, h * chh:(h + 1) * chh].rearrange("p j c -> p c j"),
                            in1=ex[:, h:h + 1, :].to_broadcast([128, chh, r]),
                            op=AT.mult)
                    nmr = pool.tile([128, CH + H], f32, tag="nmr")
                    nc.vector.tensor_reduce(out=nmr[:, 0:CH], in_=m[:],
                                            axis=mybir.AxisListType.X, op=AT.add)
                    nc.vector.tensor_reduce(out=nmr[:, CH:CH + H], in_=ex[:],
                                            axis=mybir.AxisListType.X, op=AT.add)
                    nc.vector.tensor_tensor(out=acc[:], in0=acc[:], in1=nmr[:],
                                            op=AT.add)
                # epilogue
                n0 = g * 128
                nn = min(128, PER - n0)
                rec = pool.tile([128, H], f32, tag="rec")
                nc.vector.tensor_scalar_add(rec[:], acc[:, CH:CH + H], EPS)
                nc.vector.reciprocal(rec[:], rec[:])
                o = pool.tile([128, CH], f32, tag="o")
                chh = CH // H
                for h in range(H):
                    nc.vector.tensor_tensor(
                        out=o[:, h * chh:(h + 1) * chh]
                            .rearrange("p (a c) -> p a c", a=1),
                        in0=acc[:, h * chh:(h + 1) * chh]
                            .rearrange("p (a c) -> p a c", a=1),
                        in1=rec[:, h:h + 1][:, :, None].to_broadcast([128, 1, chh]),
                        op=AT.mult)
                nc.vector.tensor_tensor(out=o[:], in0=o[:], in1=bt[:], op=AT.add)
                nc.vector.tensor_scalar_max(o[:], o[:], 0.0)
                if not last:
                    nc.sync.dma_start(out=out[n0:n0 + nn, :], in_=o[:nn])
                else:
                    yv = pool.tile([128, CH], f32, tag="yv")
                    nc.vector.tensor_tensor(out=yv[:], in0=o[:], in1=wcft[:],
                                            op=AT.mult)
                    ys = pool.tile([128, 1], f32, tag="ys")
                    nc.vector.tensor_reduce(out=ys[:], in_=yv[:],
                                            axis=mybir.AxisListType.X, op=AT.add)
                    nc.sync.dma_start(out=out[n0:n0 + nn, :], in_=ys[:nn])
    nc.compile()
    _split_waits(nc)
    return nc


def _comb(W, a_s, a_d, heads):
    W = np.asarray(W, np.float64)
    ch = W.shape[1]
    c = ch // heads
    As = np.zeros((ch, heads))
    Ad = np.zeros((ch, heads))
    a_s = np.asarray(a_s, np.float64).reshape(heads, c)
    a_d = np.asarray(a_d, np.float64).reshape(heads, c)
    for h in range(heads):
        As[h * c:(h + 1) * c, h] = a_s[h]
        Ad[h * c:(h + 1) * c, h] = a_d[h]
    return np.ascontiguousarray(np.concatenate([W, W @ As, W @ Ad], 1),
                                dtype=np.float32)


def _pad_T(a, cols):
    """a: [N, f] -> transposed+padded [f, cols] fp32 contiguous."""
    aT = np.zeros((a.shape[1], cols), np.float32)
    aT[:, :a.shape[0]] = np.asarray(a, np.float32).T
    return aT


def _run_retry(nc, in_maps, tries=4):
    from concourse.bass_utils import run_bass_kernel_spmd
    import time as _t
    for a in range(tries):
        try:
            return run_bass_kernel_spmd(nc, in_maps, core_ids=list(range(NC)))
        except Exception:
            if a == tries - 1:
                raise
            _t.sleep(3)


def kernel(x, edge_index, W1, a_src1, a_dst1, b1, W2, a_src2, a_dst2, b2, Wc, bc):

    x = np.asarray(x, np.float32)
    rounds, idx_arrays = _prep(np.asarray(edge_index))
    nfull = ((N + 127) // 128) * 128

    W1c = _comb(W1, a_src1, a_dst1, HEADS)
    W2c = _comb(W2, a_src2, a_dst2, 1)
    dummy = np.zeros((NWIN, D), np.float32)
    dummy[:, HID:HID + 2 * HEADS] = AS_DUMMY

    xT = _pad_T(x, nfull)

    nc1 = _build_layer(rounds, F_IN, HEADS, False)
    in_maps = []
    for c in range(NC):
        m = {"xT": xT,
             "xsT": np.ascontiguousarray(xT[:, c * PER:c * PER + NPAD])
             if c * PER + NPAD <= nfull else _pad_T(x[c * PER:, :], NPAD),
             "Wcomb": W1c, "dumrow": dummy,
             "bias": np.tile(np.asarray(b1, np.float32)[None, :], (128, 1)),
             "wcf": np.zeros((128, HID), np.float32)}
        for w in range(NWIN):
            m[f"idx{w}"] = idx_arrays[c][w]
        in_maps.append(m)
    res1 = _run_retry(nc1, in_maps)
    out1 = np.concatenate([res1.results[c]["out"] for c in range(NC)], 0)

    o1T = _pad_T(out1, nfull)
    nc2 = _build_layer(rounds, HID, 1, True)
    in_maps2 = []
    for c in range(NC):
        m = {"xT": o1T,
             "xsT": np.ascontiguousarray(o1T[:, c * PER:c * PER + NPAD])
             if c * PER + NPAD <= nfull else _pad_T(out1[c * PER:, :], NPAD),
             "Wcomb": W2c, "dumrow": dummy,
             "bias": np.tile(np.asarray(b2, np.float32)[None, :], (128, 1)),
             "wcf": np.tile(np.asarray(Wc, np.float32).reshape(1, HID), (128, 1))}
        for w in range(NWIN):
            m[f"idx{w}"] = idx_arrays[c][w]
        in_maps2.append(m)
    res2 = _run_retry(nc2, in_maps2)
    y = np.concatenate([res2.results[c]["out"] for c in range(NC)], 0)
    return (y + float(np.asarray(bc).ravel()[0])).astype(np.float32)


# revision 7
# speedup vs baseline: 1.2193x; 1.2193x over previous
"""2-layer GAT (PyG GATConv semantics) on 8 Trainium2 NeuronCores.

Sharding: edges partitioned by destination node (12500 dst nodes per core).
Per layer, each core:
  - builds the full node table rows [h | as | ad | pad] (512B) in its own HBM
    via PE matmul from the replicated layer input (x^T resp. out1^T),
  - edge phase: dst nodes in fixed groups of 128 (partition-aligned); each
    group's incoming edges laid out round-major per src-window (4 windows of
    25088 table rows so dma_gather's int16 indices suffice); 512B rows
    gathered per edge; softmax num/den accumulated with DVE mult+reduce.
    No max-subtraction is needed: e values are O(1) so exp() cannot overflow
    and softmax is shift-invariant. Dummy edge slots index a dummy row with
    a_src=-100 => exp contribution ~4e-8.
Two NEFF dispatches (one per GAT layer); host only reshards between them.
"""
import numpy as np

N = 100000
F_IN = 128
HID = 64
HEADS = 4
EPS = 1e-16
SLOPE = 0.2
NC = 8
PER = N // NC
NGRP = (PER + 127) // 128      # 98
NPAD = NGRP * 128              # 12544
NWIN = 4
WCAP = 25087
WROW = 25088
V = NWIN * WROW
D = 128                        # table row width in fp32 (512B)
AS_DUMMY = -100.0

_cache = {}


def _split_waits(nc):
    """This walrus encodes at most one sync-wait per instruction; move the
    excess onto InstNoOps just before the instruction (same engine order)."""
    import bass_rust
    import concourse.mybir as mybir
    n = 0
    for f in nc.m.functions:
        for bb in f.blocks:
            insts = bb.instructions
            out = []
            changed = False
            for ins in insts:
                si = ins.sync_info
                if si is not None and len(si.on_wait) > 1:
                    waits = list(si.on_wait)
                    for j, w in enumerate(waits[:-1]):
                        nop = mybir.InstNoOp(name=f"{ins.name}-ws{j}",
                                             engine=ins.engine, ins=[], outs=[])
                        nop.sync_info = bass_rust.SyncInfo(on_wait=[w],
                                                           on_update=[])
                        out.append(nop)
                    ins.sync_info = bass_rust.SyncInfo(
                        on_wait=waits[-1:], on_update=list(si.on_update))
                    changed = True
                    n += 1
                out.append(ins)
            if changed:
                bb.instructions = out
    return n


def _prep(edge_index):
    import hashlib
    key = hashlib.sha1(np.ascontiguousarray(edge_index)).hexdigest()
    if key in _cache:
        return _cache[key]
    src = np.asarray(edge_index[0], np.int64)
    dst = np.asarray(edge_index[1], np.int64)
    wsrc = np.minimum(src // WCAP, NWIN - 1)
    lsrc = src - wsrc * WCAP
    core = dst // PER
    grp = (dst % PER) // 128
    part = (dst % PER) % 128
    okey = ((core * NWIN + wsrc) * NGRP + grp) * 128 + part
    order = np.argsort(okey, kind="stable")
    ok_s = okey[order]
    ls_s = lsrc[order]
    first = np.r_[0, np.flatnonzero(np.diff(ok_s)) + 1]
    runlen = np.diff(np.r_[first, len(ok_s)])
    rank = np.arange(len(ok_s)) - np.repeat(first, runlen)
    cntd = np.zeros(NC * NWIN * NGRP * 128, np.int64)
    cntd[ok_s[first]] = runlen
    cntd = cntd.reshape(NC, NWIN, NGRP, 128)
    rounds = cntd.max(axis=3).max(axis=0).astype(np.int64)   # [NWIN, NGRP]
    tile_base = np.zeros((NWIN, NGRP), np.int64)
    for w in range(NWIN):
        tile_base[w, 1:] = np.cumsum(rounds[w])[:-1]
    c_s = ok_s // (NWIN * NGRP * 128)
    w_s = (ok_s // (NGRP * 128)) % NWIN
    g_s = (ok_s // 128) % NGRP
    p_s = ok_s % 128
    t_s = tile_base[w_s, g_s] + rank
    idx_arrays = []
    for c in range(NC):
        per_w = []
        for w in range(NWIN):
            ntile = int(rounds[w].sum())
            arr = np.full((ntile, 128), WCAP, np.int16)
            m = (c_s == c) & (w_s == w)
            arr[t_s[m], p_s[m]] = ls_s[m].astype(np.int16)
            wr = arr.reshape(ntile, 8, 16).transpose(2, 0, 1).reshape(16, ntile * 8)
            per_w.append(np.ascontiguousarray(np.tile(wr, (8, 1)), dtype=np.int16))
        idx_arrays.append(per_w)
    res = (rounds, idx_arrays)
    _cache[key] = res
    return res


def _build_layer(rounds, f_in, heads, last):
    import concourse.bacc as bacc
    import concourse.mybir as mybir
    from concourse.tile import TileContext

    H = heads
    CH = 64
    f32 = mybir.dt.float32
    AT = mybir.AluOpType
    nc = bacc.Bacc("TRN2")
    nfull = ((N + 127) // 128) * 128
    xT = nc.dram_tensor("xT", [f_in, nfull], f32, kind="ExternalInput")
    xsT = nc.dram_tensor("xsT", [f_in, NPAD], f32, kind="ExternalInput")
    Wcomb = nc.dram_tensor("Wcomb", [f_in, CH + 2 * H], f32, kind="ExternalInput")
    dumrow = nc.dram_tensor("dumrow", [NWIN, D], f32, kind="ExternalInput")
    bias = nc.dram_tensor("bias", [128, CH], f32, kind="ExternalInput")
    wcf = nc.dram_tensor("wcf", [128, CH], f32, kind="ExternalInput")
    idxs = [nc.dram_tensor(f"idx{w}", [128, int(rounds[w].sum()) * 8],
                           mybir.dt.int16, kind="ExternalInput")
            for w in range(NWIN)]
    table = nc.dram_tensor("table", [V, D], f32)
    out = nc.dram_tensor("out", [PER, 1 if last else CH], f32,
                         kind="ExternalOutput")

    with TileContext(nc) as tc:
        with tc.tile_pool(name="const", bufs=1) as cpool, \
             tc.tile_pool(name="work", bufs=3) as pool, \
             tc.tile_pool(name="gb", bufs=3) as gpool, \
             tc.tile_pool(name="acc", bufs=2) as apool, \
             tc.tile_pool(name="ps", bufs=4, space="PSUM") as ppool, \
             tc.tile_pool(name="adp", bufs=1) as adpool:
            wct = cpool.tile([f_in, CH + 2 * H], f32)
            nc.sync.dma_start(out=wct[:], in_=Wcomb[:, :])
            bt = cpool.tile([128, CH], f32)
            nc.sync.dma_start(out=bt[:], in_=bias[:, :])
            wcft = cpool.tile([128, CH], f32)
            nc.sync.dma_start(out=wcft[:], in_=wcf[:, :])
            ad_arr = adpool.tile([128, NGRP * H], f32)

            table_writes = []
            for w in range(NWIN):
                dt_ = pool.tile([1, D], f32, tag="dt")
                nc.sync.dma_start(out=dt_[:], in_=dumrow[w:w + 1, :])
                table_writes.append(
                    nc.sync.dma_start(out=table[(w + 1) * WROW - 1:(w + 1) * WROW, :],
                                      in_=dt_[:]))

            # full-table prologue
            for ci in range(nfull // 128):
                n0 = ci * 128
                lx = pool.tile([f_in, 128], f32, tag="lx")
                nc.sync.dma_start(out=lx[:], in_=xT[:, n0:n0 + 128])
                ps = ppool.tile([128, CH + 2 * H], f32)
                nc.tensor.matmul(ps[:], lhsT=lx[:], rhs=wct[:],
                                 start=True, stop=True)
                st = pool.tile([128, D], f32, tag="st")
                nc.scalar.copy(out=st[:, :CH + H], in_=ps[:, :CH + H])
                runs = []
                a = n0
                while a < n0 + 128:
                    w = min(a // WCAP, NWIN - 1)
                    b = min(n0 + 128, (w + 1) * WCAP) if w < NWIN - 1 else n0 + 128
                    runs.append((a, b, a + w))
                    a = b
                for (a, b, row) in runs:
                    table_writes.append(
                        nc.sync.dma_start(out=table[row:row + (b - a), :],
                                          in_=st[a - n0:b - n0, :]))

            # per-core ad (own dst slice)
            for g in range(NGRP):
                lxs = pool.tile([f_in, 128], f32, tag="lxs")
                nc.sync.dma_start(out=lxs[:], in_=xsT[:, g * 128:(g + 1) * 128])
                ps2 = ppool.tile([128, H], f32, tag="ps2")
                nc.tensor.matmul(ps2[:], lhsT=lxs[:], rhs=wct[:, CH + H:CH + 2 * H],
                                 start=True, stop=True)
                nc.vector.tensor_copy(out=ad_arr[:, g * H:(g + 1) * H], in_=ps2[:])

            # fence: Tile does not track DRAM RAW deps; gathers must wait
            # for all table writes.
            from concourse.tile_rust import add_dep_helper
            fence_t = pool.tile([1, 4], f32, tag="fence")
            fence = nc.vector.memset(fence_t[:], 0.0)
            for wd in table_writes:
                add_dep_helper(fence.ins, wd.ins, reason="table RAW fence")

            # edge phase: per group, per window band
            tile_base = np.zeros((NWIN, NGRP), np.int64)
            for w in range(NWIN):
                tile_base[w, 1:] = np.cumsum(rounds[w])[:-1]
            for g in range(NGRP):
                acc = apool.tile([128, CH + H], f32, tag="acc")
                nc.vector.memset(acc[:], 0.0)
                for w in range(NWIN):
                    r = int(rounds[w, g])
                    if r == 0:
                        continue
                    t0 = int(tile_base[w, g])
                    it = pool.tile([128, r * 8], mybir.dt.int16, tag="it")
                    nc.sync.dma_start(out=it[:], in_=idxs[w][:, t0 * 8:(t0 + r) * 8])
                    gb = gpool.tile([128, r, D], f32, tag="gb")
                    gth = nc.gpsimd.dma_gather(gb[:], table[w * WROW:(w + 1) * WROW, :],
                                               it[:], r * 128, r * 128, D,
                                               single_packet=False)
                    add_dep_helper(gth.ins, fence.ins, reason="table RAW fence")
                    ex = pool.tile([128, H, r], f32, tag="ex")
                    for h in range(H):
                        nc.vector.tensor_tensor(
                            out=ex[:, h:h + 1, :],
                            in0=gb[:, :, CH + h:CH + h + 1].rearrange("p j c -> p c j"),
                            in1=ad_arr[:, g * H + h:g * H + h + 1][:, :, None]
                                .to_broadcast([128, 1, r]),
                            op=AT.add)
                    exf = ex[:].rearrange("p h j -> p (h j)")
                    lr = pool.tile([128, H * r], f32, tag="lr")
                    nc.vector.tensor_scalar_mul(lr[:], exf, SLOPE)
                    nc.vector.tensor_tensor(out=lr[:], in0=lr[:], in1=exf, op=AT.max)
                    nc.scalar.activation(exf, lr[:], mybir.ActivationFunctionType.Exp)
                    m = pool.tile([128, CH, r], f32, tag="m")
                    chh = CH // H
                    for h in range(H):
                        nc.vector.tensor_tensor(
                            out=m[:, h * chh:(h + 1) * chh, :],
                            in0=gb[:, :, h * chh:(h + 1) * chh].rearrange("p j c -> p c j"),
                            in1=ex[:, h:h + 1, :].to_broadcast([128, chh, r]),
                            op=AT.mult)
                    nmr = pool.tile([128, CH + H], f32, tag="nmr")
                    nc.vector.tensor_reduce(out=nmr[:, 0:CH], in_=m[:],
                                            axis=mybir.AxisListType.X, op=AT.add)
                    nc.vector.tensor_reduce(out=nmr[:, CH:CH + H], in_=ex[:],
                                            axis=mybir.AxisListType.X, op=AT.add)
                    nc.vector.tensor_tensor(out=acc[:], in0=acc[:], in1=nmr[:],
                                            op=AT.add)
                # epilogue
                n0 = g * 128
                nn = min(128, PER - n0)
                rec = pool.tile([128, H], f32, tag="rec")
                nc.vector.tensor_scalar_add(rec[:], acc[:, CH:CH + H], EPS)
                nc.vector.reciprocal(rec[:], rec[:])
                o = pool.tile([128, CH], f32, tag="o")
                chh = CH // H
                for h in range(H):
                    nc.vector.tensor_tensor(
                        out=o[:, h * chh:(h + 1) * chh]
                            .rearrange("p (a c) -> p a c", a=1),
                        in0=acc[:, h * chh:(h + 1) * chh]
                            .rearrange("p (a c) -> p a c", a=1),
                        in1=rec[:, h:h + 1][:, :, None].to_broadcast([128, 1, chh]),
                        op=AT.mult)
                nc.vector.tensor_tensor(out=o[:], in0=o[:], in1=bt[:], op=AT.add)
                nc.vector.tensor_scalar_max(o[:], o[:], 0.0)
                if not last:
                    nc.sync.dma_start(out=out[n0:n0 + nn, :], in_=o[:nn])
                else:
                    yv = pool.tile([128, CH], f32, tag="yv")
                    nc.vector.tensor_tensor(out=yv[:], in0=o[:], in1=wcft[:],
                                            op=AT.mult)
                    ys = pool.tile([128, 1], f32, tag="ys")
                    nc.vector.tensor_reduce(out=ys[:], in_=yv[:],
                                            axis=mybir.AxisListType.X, op=AT.add)
                    nc.sync.dma_start(out=out[n0:n0 + nn, :], in_=ys[:nn])
    nc.compile()
    _split_waits(nc)
    return nc


def _comb(W, a_s, a_d, heads):
    W = np.asarray(W, np.float64)
    ch = W.shape[1]
    c = ch // heads
    As = np.zeros((ch, heads))
    Ad = np.zeros((ch, heads))
    a_s = np.asarray(a_s, np.float64).reshape(heads, c)
    a_d = np.asarray(a_d, np.float64).reshape(heads, c)
    for h in range(heads):
        As[h * c:(h + 1) * c, h] = a_s[h]
        Ad[h * c:(h + 1) * c, h] = a_d[h]
    return np.ascontiguousarray(np.concatenate([W, W @ As, W @ Ad], 1),
                                dtype=np.float32)


def _pad_T(a, cols):
    """a: [N, f] -> transposed+padded [f, cols] fp32 contiguous."""
    aT = np.zeros((a.shape[1], cols), np.float32)
    aT[:, :a.shape[0]] = np.asarray(a, np.float32).T
    return aT


def _run_retry(nc, in_maps, tries=4):
    from concourse.bass_utils import run_bass_kernel_spmd
    import time as _t
    for a in range(tries):
        try:
            return run_bass_kernel_spmd(nc, in_maps, core_ids=list(range(NC)))
        except Exception:
            if a == tries - 1:
                raise
            _t.sleep(3)


def kernel(x, edge_index, W1, a_src1, a_dst1, b1, W2, a_src2, a_dst2, b2, Wc, bc):

    x = np.asarray(x, np.float32)
    rounds, idx_arrays = _prep(np.asarray(edge_index))
    nfull = ((N + 127) // 128) * 128
    mkey = ("modules", rounds.tobytes())
    if mkey not in _cache:
        _cache[mkey] = (_build_layer(rounds, F_IN, HEADS, False),
                        _build_layer(rounds, HID, 1, True))
    nc1, nc2 = _cache[mkey]

    W1c = _comb(W1, a_src1, a_dst1, HEADS)
    W2c = _comb(W2, a_src2, a_dst2, 1)
    dummy = np.zeros((NWIN, D), np.float32)
    dummy[:, HID:HID + 2 * HEADS] = AS_DUMMY

    xT = _pad_T(x, nfull)

    in_maps = []
    for c in range(NC):
        m = {"xT": xT,
             "xsT": np.ascontiguousarray(xT[:, c * PER:c * PER + NPAD])
             if c * PER + NPAD <= nfull else _pad_T(x[c * PER:, :], NPAD),
             "Wcomb": W1c, "dumrow": dummy,
             "bias": np.tile(np.asarray(b1, np.float32)[None, :], (128, 1)),
             "wcf": np.zeros((128, HID), np.float32)}
        for w in range(NWIN):
            m[f"idx{w}"] = idx_arrays[c][w]
        in_maps.append(m)
    res1 = _run_retry(nc1, in_maps)
    out1 = np.concatenate([res1.results[c]["out"] for c in range(NC)], 0)

    o1T = _pad_T(out1, nfull)
    in_maps2 = []
    for c in range(NC):
        m = {"xT": o1T,
             "xsT": np.ascontiguousarray(o1T[:, c * PER:c * PER + NPAD])
             if c * PER + NPAD <= nfull else _pad_T(out1[c * PER:, :], NPAD),
             "Wcomb": W2c, "dumrow": dummy,
             "bias": np.tile(np.asarray(b2, np.float32)[None, :], (128, 1)),
             "wcf": np.tile(np.asarray(Wc, np.float32).reshape(1, HID), (128, 1))}
        for w in range(NWIN):
            m[f"idx{w}"] = idx_arrays[c][w]
        in_maps2.append(m)
    res2 = _run_retry(nc2, in_maps2)
    y = np.concatenate([res2.results[c]["out"] for c in range(NC)], 0)
    return (y + float(np.asarray(bc).ravel()[0])).astype(np.float32)
